# revision 20
# baseline (speedup 1.0000x reference)
# Causal Haar DWT on Trainium2, SPMD across 8 NeuronCores.
#
# reference: p = [0, x_0 .. x_{L-1}] (per batch, per channel);
#   lo[t] = p[2t] + p[2t+1],  hi[t] = p[2t] - p[2t+1]
# i.e. with a(t) = x[2t-1] (zero for t=0) and b(t) = x[2t]:
#   lo = a + b, hi = a - b.
#
# Layout trick: for t >= 1 the source rows (2t-1, 2t) are ADJACENT in memory,
# so the pair chunk for pairs [t0, t0+k) is one contiguous run starting at row
# 2*t0-1. Each SBUF partition loads PAIRS_PER_PART pairs = 2*PAIRS_PER_PART
# contiguous rows via a single large DMA. The t=0 pair (causal zero pad) is
# handled by a one-shot mini-pass: lo[:,0,:] is a pure DMA copy of x[:,0,:]
# and hi[:,0,:] is a single negate. The last bulk tile starts one pair early
# (recomputing one pair already written) so every tile is a full, uniform
# 128-partition tile with exactly one DMA writer — this keeps every DVE
# instruction at <=2 semaphore waits (the core_v3 codegen limit).
import numpy as np

B, L, C = 32, 4096, 512
NCORES = 8
BL = B // NCORES            # batches per core
L2 = L // 2                 # output time steps
P = 128                     # SBUF partitions
PAIRS_PER_PART = 8          # Haar pairs per partition (4 MB loads, 2 MB stores)
TILE_PAIRS = P * PAIRS_PER_PART          # pairs per tile
ROWS_PER_PART = 2 * PAIRS_PER_PART       # input rows per partition
FREE_IN = ROWS_PER_PART * C              # f32 per partition in
FREE_OUT = PAIRS_PER_PART * C            # f32 per partition out


def _legalize_waits(nc, max_waits=1):
    """Hoist excess per-instruction semaphore waits into standalone
    EventSemaphore instructions.

    This walrus build rejects instructions whose embedded on_wait list
    exceeds the ISA wait-slot capacity ("Too many sync wait commands";
    observed: 2 waits on an SP DMACopy and 3 on a DVE TensorTensor both
    fail). The engine executes its instruction stream in order, so a
    wait-only EventSemaphore placed immediately before the instruction is
    semantically identical to an embedded wait.
    """
    import concourse.mybir as mybir

    n = 0
    for fn in nc.m.functions:
        for blk in fn.blocks:
            out = []
            for inst in blk.instructions:
                si = inst.sync_info
                if (
                    si is not None
                    and si.on_wait
                    and len(si.on_wait) > max_waits
                    and inst.opcode != "EventSemaphore"
                ):
                    waits = list(si.on_wait)
                    for w in waits[max_waits:]:
                        n += 1
                        ev = mybir.InstEventSemaphore(
                            name=f"legalize_wait_{n}", engine=inst.engine
                        )
                        ev.sync_info = mybir.SyncInfo(on_wait=[w], on_update=[])
                        nc.inst_map[ev.name] = ev
                        out.append(ev)
                    inst.sync_info = mybir.SyncInfo(
                        on_wait=waits[:max_waits],
                        on_update=list(si.on_update or []),
                    )
                out.append(inst)
            blk.instructions[:] = out
    return nc


def build(
    bl=BL,
    l=L,
    pairs_per_part=PAIRS_PER_PART,
    bufs=2,
    out_engine="sync",
    in_engine="sync",
    hi_engine=None,
    store_cpp=4,
    out_bufs=3,
):
    import concourse.bass as bass
    import concourse.mybir as mybir
    from concourse.tile import TileContext

    tile_pairs = P * pairs_per_part
    rows_per_part = 2 * pairs_per_part
    free_in = rows_per_part * C
    free_out = pairs_per_part * C

    l2 = l // 2
    ntiles = l2 // tile_pairs
    assert l2 % tile_pairs == 0
    # the last tile starts one pair early (overlap); with a single tile that
    # would underflow into row -1
    assert ntiles >= 2

    f32 = mybir.dt.float32
    nc = bass.Bass()
    x = nc.dram_tensor("x", [bl, l, C], f32, kind="ExternalInput")
    lo = nc.dram_tensor("lo", [bl, l2, C], f32, kind="ExternalOutput")
    hi = nc.dram_tensor("hi", [bl, l2, C], f32, kind="ExternalOutput")

    cpp = store_cpp or pairs_per_part  # compute/store chunk (pairs/partition)
    assert pairs_per_part % cpp == 0

    with TileContext(nc) as tc:
        with (
            tc.tile_pool(name="io", bufs=bufs) as pool,
            tc.tile_pool(name="outs", bufs=out_bufs or bufs) as opool,
        ):
            out_eng = getattr(nc, out_engine)
            in_eng = getattr(nc, in_engine)
            hi_eng = getattr(nc, hi_engine) if hi_engine else out_eng

            def emit_tile(b, p0, ppp):
                """Process pairs [p0, p0+128*ppp) of batch b (p0 >= 1).

                One big load; compute+stores run in `cpp`-pair chunks along
                the free dim so stores begin before the whole tile is
                reduced (shorter pipeline tail).
                """
                rpp = 2 * ppp
                r0 = 2 * p0 - 1
                in_tile = pool.tile([P, rpp * C], f32, tag=f"in{ppp}")
                in_eng.dma_start(
                    out=in_tile[:, :],
                    in_=x[b, r0 : r0 + P * rpp, :].rearrange(
                        "(n r) c -> n (r c)", r=rpp
                    ),
                )
                ccpp = min(cpp, ppp)
                v = in_tile[:, :].rearrange("p (n two c) -> p n two c", two=2, c=C)
                for k in range(ppp // ccpp):
                    lo_t = opool.tile([P, ccpp * C], f32, tag=f"lo{ccpp}")
                    hi_t = opool.tile([P, ccpp * C], f32, tag=f"hi{ccpp}")
                    a = v[:, k * ccpp : (k + 1) * ccpp, 0:1, :]
                    bb = v[:, k * ccpp : (k + 1) * ccpp, 1:2, :]
                    lo_v = lo_t[:, :].rearrange("p (n o c) -> p n o c", o=1, c=C)
                    hi_v = hi_t[:, :].rearrange("p (n o c) -> p n o c", o=1, c=C)
                    nc.vector.tensor_add(lo_v, a, bb)
                    nc.vector.tensor_sub(hi_v, a, bb)
                    # chunk k of partition p covers pairs p0 + p*ppp + k*ccpp
                    # + [0, ccpp): DRAM runs of ccpp rows, stride ppp rows.
                    out_eng.dma_start(
                        out=lo[b, p0 : p0 + P * ppp, :]
                        .rearrange("(n r) c -> n (r c)", r=ppp)[
                            :, k * ccpp * C : (k * ccpp + ccpp) * C
                        ],
                        in_=lo_t[:, :],
                    )
                    hi_eng.dma_start(
                        out=hi[b, p0 : p0 + P * ppp, :]
                        .rearrange("(n r) c -> n (r c)", r=ppp)[
                            :, k * ccpp * C : (k * ccpp + ccpp) * C
                        ],
                        in_=hi_t[:, :],
                    )

            # pair-0 mini-pass (the causal zero pad): lo[:,0,:] = x[:,0,:],
            # hi[:,0,:] = -x[:,0,:].
            t0_in = pool.tile([bl, C], f32, tag="t0in")
            t0_hi = pool.tile([bl, C], f32, tag="t0hi")
            nc.sync.dma_start(out=t0_in[:, :], in_=x[:, 0, :])
            nc.vector.tensor_scalar_mul(t0_hi[:, :], t0_in[:, :], -1.0)
            out_eng.dma_start(out=lo[:, 0, :], in_=t0_in[:, :])
            out_eng.dma_start(out=hi[:, 0, :], in_=t0_hi[:, :])

            for b in range(bl):
                for i in range(ntiles):
                    p0 = i * tile_pairs + 1
                    if i == ntiles - 1:
                        p0 -= 1  # overlap one pair so the tile stays full
                    emit_tile(b, p0, pairs_per_part)
    return _legalize_waits(nc)


_nc_cache = None


def _get_nc():
    global _nc_cache
    if _nc_cache is None:
        _nc_cache = build()
    return _nc_cache


def run(x, trace=False, trace_cores=None):
    """x: [B, L, C] float32. Returns (lo, hi, BassKernelResults)."""
    from concourse.bass_utils import run_bass_kernel_spmd

    x = np.ascontiguousarray(np.asarray(x), dtype=np.float32)
    assert x.shape == (B, L, C)
    nc = _get_nc()
    in_maps = [{"x": x[i * BL : (i + 1) * BL]} for i in range(NCORES)]
    kwargs = {}
    if trace_cores is not None:
        kwargs["trace_cores"] = trace_cores
    r = run_bass_kernel_spmd(nc, in_maps, list(range(NCORES)), trace=trace, **kwargs)
    lo = np.concatenate([r.results[i]["lo"] for i in range(NCORES)], axis=0)
    hi = np.concatenate([r.results[i]["hi"] for i in range(NCORES)], axis=0)
    return lo, hi, r


def kernel(**inputs):
    lo, hi, _ = run(inputs["x"])
    return lo, hi


# revision 23
# speedup vs baseline: 1.0038x; 1.0038x over previous
# Causal Haar DWT on Trainium2, SPMD across 8 NeuronCores.
#
# reference: p = [0, x_0 .. x_{L-1}] (per batch, per channel);
#   lo[t] = p[2t] + p[2t+1],  hi[t] = p[2t] - p[2t+1]
# i.e. with a(t) = x[2t-1] (zero for t=0) and b(t) = x[2t]:
#   lo = a + b, hi = a - b.
#
# Layout trick: for t >= 1 the source rows (2t-1, 2t) are ADJACENT in memory,
# so the pair chunk for pairs [t0, t0+k) is one contiguous run starting at row
# 2*t0-1. Each SBUF partition loads PAIRS_PER_PART pairs = 2*PAIRS_PER_PART
# contiguous rows via a single large DMA. The t=0 pair (causal zero pad) is
# handled by a one-shot mini-pass: lo[:,0,:] is a pure DMA copy of x[:,0,:]
# and hi[:,0,:] is a single negate. The last bulk tile starts one pair early
# (recomputing one pair already written) so every tile is a full, uniform
# 128-partition tile with exactly one DMA writer — this keeps every DVE
# instruction at <=2 semaphore waits (the core_v3 codegen limit).
import numpy as np

B, L, C = 32, 4096, 512
NCORES = 8
BL = B // NCORES            # batches per core
L2 = L // 2                 # output time steps
P = 128                     # SBUF partitions
PAIRS_PER_PART = 8          # Haar pairs per partition (4 MB loads, 2 MB stores)
TILE_PAIRS = P * PAIRS_PER_PART          # pairs per tile
ROWS_PER_PART = 2 * PAIRS_PER_PART       # input rows per partition
FREE_IN = ROWS_PER_PART * C              # f32 per partition in
FREE_OUT = PAIRS_PER_PART * C            # f32 per partition out


def _legalize_waits(nc, max_waits=1):
    """Hoist excess per-instruction semaphore waits into standalone
    EventSemaphore instructions.

    This walrus build rejects instructions whose embedded on_wait list
    exceeds the ISA wait-slot capacity ("Too many sync wait commands";
    observed: 2 waits on an SP DMACopy and 3 on a DVE TensorTensor both
    fail). The engine executes its instruction stream in order, so a
    wait-only EventSemaphore placed immediately before the instruction is
    semantically identical to an embedded wait.
    """
    import concourse.mybir as mybir

    n = 0
    for fn in nc.m.functions:
        for blk in fn.blocks:
            out = []
            for inst in blk.instructions:
                si = inst.sync_info
                if (
                    si is not None
                    and si.on_wait
                    and len(si.on_wait) > max_waits
                    and inst.opcode != "EventSemaphore"
                ):
                    waits = list(si.on_wait)
                    for w in waits[max_waits:]:
                        n += 1
                        ev = mybir.InstEventSemaphore(
                            name=f"legalize_wait_{n}", engine=inst.engine
                        )
                        ev.sync_info = mybir.SyncInfo(on_wait=[w], on_update=[])
                        nc.inst_map[ev.name] = ev
                        out.append(ev)
                    inst.sync_info = mybir.SyncInfo(
                        on_wait=waits[:max_waits],
                        on_update=list(si.on_update or []),
                    )
                out.append(inst)
            blk.instructions[:] = out
    return nc


def build(
    bl=BL,
    l=L,
    pairs_per_part=PAIRS_PER_PART,
    bufs=2,
    out_engine="sync",
    in_engine="sync",
    hi_engine=None,
    store_cpp=4,
    out_bufs=3,
    in_cpp=4,
):
    import concourse.bass as bass
    import concourse.mybir as mybir
    from concourse.tile import TileContext

    tile_pairs = P * pairs_per_part
    rows_per_part = 2 * pairs_per_part
    free_in = rows_per_part * C
    free_out = pairs_per_part * C

    l2 = l // 2
    ntiles = l2 // tile_pairs
    assert l2 % tile_pairs == 0
    # the last tile starts one pair early (overlap); with a single tile that
    # would underflow into row -1
    assert ntiles >= 2

    f32 = mybir.dt.float32
    nc = bass.Bass()
    x = nc.dram_tensor("x", [bl, l, C], f32, kind="ExternalInput")
    lo = nc.dram_tensor("lo", [bl, l2, C], f32, kind="ExternalOutput")
    hi = nc.dram_tensor("hi", [bl, l2, C], f32, kind="ExternalOutput")

    cpp = store_cpp or pairs_per_part  # compute/store chunk (pairs/partition)
    assert pairs_per_part % cpp == 0

    with TileContext(nc) as tc:
        with (
            tc.tile_pool(name="io", bufs=bufs) as pool,
            tc.tile_pool(name="outs", bufs=out_bufs or bufs) as opool,
        ):
            out_eng = getattr(nc, out_engine)
            in_eng = getattr(nc, in_engine)
            hi_eng = getattr(nc, hi_engine) if hi_engine else out_eng

            def emit_tile(b, p0, ppp):
                """Process pairs [p0, p0+128*ppp) of batch b (p0 >= 1).

                One big load; compute+stores run in `cpp`-pair chunks along
                the free dim so stores begin before the whole tile is
                reduced (shorter pipeline tail).
                """
                rpp = 2 * ppp
                r0 = 2 * p0 - 1
                in_tile = pool.tile([P, rpp * C], f32, tag=f"in{ppp}")
                src = x[b, r0 : r0 + P * rpp, :].rearrange(
                    "(n r) c -> n (r c)", r=rpp
                )
                icpp = min(in_cpp or ppp, ppp)
                for j in range(ppp // icpp):
                    c0, c1 = j * 2 * icpp * C, (j + 1) * 2 * icpp * C
                    in_eng.dma_start(out=in_tile[:, c0:c1], in_=src[:, c0:c1])
                ccpp = min(cpp, ppp)
                v = in_tile[:, :].rearrange("p (n two c) -> p n two c", two=2, c=C)
                for k in range(ppp // ccpp):
                    lo_t = opool.tile([P, ccpp * C], f32, tag=f"lo{ccpp}")
                    hi_t = opool.tile([P, ccpp * C], f32, tag=f"hi{ccpp}")
                    a = v[:, k * ccpp : (k + 1) * ccpp, 0:1, :]
                    bb = v[:, k * ccpp : (k + 1) * ccpp, 1:2, :]
                    lo_v = lo_t[:, :].rearrange("p (n o c) -> p n o c", o=1, c=C)
                    hi_v = hi_t[:, :].rearrange("p (n o c) -> p n o c", o=1, c=C)
                    nc.vector.tensor_add(lo_v, a, bb)
                    nc.vector.tensor_sub(hi_v, a, bb)
                    # chunk k of partition p covers pairs p0 + p*ppp + k*ccpp
                    # + [0, ccpp): DRAM runs of ccpp rows, stride ppp rows.
                    out_eng.dma_start(
                        out=lo[b, p0 : p0 + P * ppp, :]
                        .rearrange("(n r) c -> n (r c)", r=ppp)[
                            :, k * ccpp * C : (k * ccpp + ccpp) * C
                        ],
                        in_=lo_t[:, :],
                    )
                    hi_eng.dma_start(
                        out=hi[b, p0 : p0 + P * ppp, :]
                        .rearrange("(n r) c -> n (r c)", r=ppp)[
                            :, k * ccpp * C : (k * ccpp + ccpp) * C
                        ],
                        in_=hi_t[:, :],
                    )

            # pair-0 mini-pass (the causal zero pad): lo[:,0,:] = x[:,0,:],
            # hi[:,0,:] = -x[:,0,:].
            t0_in = pool.tile([bl, C], f32, tag="t0in")
            t0_hi = pool.tile([bl, C], f32, tag="t0hi")
            nc.sync.dma_start(out=t0_in[:, :], in_=x[:, 0, :])
            nc.vector.tensor_scalar_mul(t0_hi[:, :], t0_in[:, :], -1.0)
            out_eng.dma_start(out=lo[:, 0, :], in_=t0_in[:, :])
            out_eng.dma_start(out=hi[:, 0, :], in_=t0_hi[:, :])

            for b in range(bl):
                for i in range(ntiles):
                    p0 = i * tile_pairs + 1
                    if i == ntiles - 1:
                        p0 -= 1  # overlap one pair so the tile stays full
                    emit_tile(b, p0, pairs_per_part)
    return _legalize_waits(nc)


_nc_cache = None


def _get_nc():
    global _nc_cache
    if _nc_cache is None:
        _nc_cache = build()
    return _nc_cache


def run(x, trace=False, trace_cores=None):
    """x: [B, L, C] float32. Returns (lo, hi, BassKernelResults)."""
    from concourse.bass_utils import run_bass_kernel_spmd

    x = np.ascontiguousarray(np.asarray(x), dtype=np.float32)
    assert x.shape == (B, L, C)
    nc = _get_nc()
    in_maps = [{"x": x[i * BL : (i + 1) * BL]} for i in range(NCORES)]
    kwargs = {}
    if trace_cores is not None:
        kwargs["trace_cores"] = trace_cores
    r = run_bass_kernel_spmd(nc, in_maps, list(range(NCORES)), trace=trace, **kwargs)
    lo = np.concatenate([r.results[i]["lo"] for i in range(NCORES)], axis=0)
    hi = np.concatenate([r.results[i]["hi"] for i in range(NCORES)], axis=0)
    return lo, hi, r


def kernel(**inputs):
    lo, hi, _ = run(inputs["x"])
    return lo, hi


# revision 25
# speedup vs baseline: 1.0286x; 1.0247x over previous
# Causal Haar DWT on Trainium2, SPMD across 8 NeuronCores.
#
# reference: p = [0, x_0 .. x_{L-1}] (per batch, per channel);
#   lo[t] = p[2t] + p[2t+1],  hi[t] = p[2t] - p[2t+1]
# i.e. with a(t) = x[2t-1] (zero for t=0) and b(t) = x[2t]:
#   lo = a + b, hi = a - b.
#
# Layout trick: for t >= 1 the source rows (2t-1, 2t) are ADJACENT in memory,
# so the pair chunk for pairs [t0, t0+k) is one contiguous run starting at row
# 2*t0-1. Each SBUF partition loads PAIRS_PER_PART pairs = 2*PAIRS_PER_PART
# contiguous rows via a single large DMA. The t=0 pair (causal zero pad) is
# handled by a one-shot mini-pass: lo[:,0,:] is a pure DMA copy of x[:,0,:]
# and hi[:,0,:] is a single negate. The last bulk tile starts one pair early
# (recomputing one pair already written) so every tile is a full, uniform
# 128-partition tile with exactly one DMA writer — this keeps every DVE
# instruction at <=2 semaphore waits (the core_v3 codegen limit).
import numpy as np

B, L, C = 32, 4096, 512
NCORES = 8
BL = B // NCORES            # batches per core
L2 = L // 2                 # output time steps
P = 128                     # SBUF partitions
PAIRS_PER_PART = 8          # Haar pairs per partition (4 MB loads, 2 MB stores)
TILE_PAIRS = P * PAIRS_PER_PART          # pairs per tile
ROWS_PER_PART = 2 * PAIRS_PER_PART       # input rows per partition
FREE_IN = ROWS_PER_PART * C              # f32 per partition in
FREE_OUT = PAIRS_PER_PART * C            # f32 per partition out


def _legalize_waits(nc, max_waits=1):
    """Hoist excess per-instruction semaphore waits into standalone
    EventSemaphore instructions.

    This walrus build rejects instructions whose embedded on_wait list
    exceeds the ISA wait-slot capacity ("Too many sync wait commands";
    observed: 2 waits on an SP DMACopy and 3 on a DVE TensorTensor both
    fail). The engine executes its instruction stream in order, so a
    wait-only EventSemaphore placed immediately before the instruction is
    semantically identical to an embedded wait.
    """
    import concourse.mybir as mybir

    n = 0
    for fn in nc.m.functions:
        for blk in fn.blocks:
            out = []
            for inst in blk.instructions:
                si = inst.sync_info
                if (
                    si is not None
                    and si.on_wait
                    and len(si.on_wait) > max_waits
                    and inst.opcode != "EventSemaphore"
                ):
                    waits = list(si.on_wait)
                    for w in waits[max_waits:]:
                        n += 1
                        ev = mybir.InstEventSemaphore(
                            name=f"legalize_wait_{n}", engine=inst.engine
                        )
                        ev.sync_info = mybir.SyncInfo(on_wait=[w], on_update=[])
                        nc.inst_map[ev.name] = ev
                        out.append(ev)
                    inst.sync_info = mybir.SyncInfo(
                        on_wait=waits[:max_waits],
                        on_update=list(si.on_update or []),
                    )
                out.append(inst)
            blk.instructions[:] = out
    return nc


def build(
    bl=BL,
    l=L,
    pairs_per_part=PAIRS_PER_PART,
    bufs=2,
    out_engine="sync",
    in_engine="sync",
    hi_engine=None,
    store_cpp=4,
    out_bufs=4,
    in_cpp=4,
):
    import concourse.bass as bass
    import concourse.mybir as mybir
    from concourse.tile import TileContext

    tile_pairs = P * pairs_per_part
    rows_per_part = 2 * pairs_per_part
    free_in = rows_per_part * C
    free_out = pairs_per_part * C

    l2 = l // 2
    ntiles = l2 // tile_pairs
    assert l2 % tile_pairs == 0
    # the last tile starts one pair early (overlap); with a single tile that
    # would underflow into row -1
    assert ntiles >= 2

    f32 = mybir.dt.float32
    nc = bass.Bass()
    x = nc.dram_tensor("x", [bl, l, C], f32, kind="ExternalInput")
    lo = nc.dram_tensor("lo", [bl, l2, C], f32, kind="ExternalOutput")
    hi = nc.dram_tensor("hi", [bl, l2, C], f32, kind="ExternalOutput")

    cpp = store_cpp or pairs_per_part  # compute/store chunk (pairs/partition)
    assert pairs_per_part % cpp == 0

    with TileContext(nc) as tc:
        with (
            tc.tile_pool(name="io", bufs=bufs) as pool,
            tc.tile_pool(name="outs", bufs=out_bufs or bufs) as opool,
        ):
            out_eng = getattr(nc, out_engine)
            in_eng = getattr(nc, in_engine)
            hi_eng = getattr(nc, hi_engine) if hi_engine else out_eng

            def emit_tile(b, p0, ppp):
                """Process pairs [p0, p0+128*ppp) of batch b (p0 >= 1).

                One big load; compute+stores run in `cpp`-pair chunks along
                the free dim so stores begin before the whole tile is
                reduced (shorter pipeline tail).
                """
                rpp = 2 * ppp
                r0 = 2 * p0 - 1
                in_tile = pool.tile([P, rpp * C], f32, tag=f"in{ppp}")
                src = x[b, r0 : r0 + P * rpp, :].rearrange(
                    "(n r) c -> n (r c)", r=rpp
                )
                icpp = min(in_cpp or ppp, ppp)
                for j in range(ppp // icpp):
                    c0, c1 = j * 2 * icpp * C, (j + 1) * 2 * icpp * C
                    in_eng.dma_start(out=in_tile[:, c0:c1], in_=src[:, c0:c1])
                ccpp = min(cpp, ppp)
                v = in_tile[:, :].rearrange("p (n two c) -> p n two c", two=2, c=C)
                for k in range(ppp // ccpp):
                    lo_t = opool.tile([P, ccpp * C], f32, tag=f"lo{ccpp}")
                    hi_t = opool.tile([P, ccpp * C], f32, tag=f"hi{ccpp}")
                    a = v[:, k * ccpp : (k + 1) * ccpp, 0:1, :]
                    bb = v[:, k * ccpp : (k + 1) * ccpp, 1:2, :]
                    lo_v = lo_t[:, :].rearrange("p (n o c) -> p n o c", o=1, c=C)
                    hi_v = hi_t[:, :].rearrange("p (n o c) -> p n o c", o=1, c=C)
                    nc.vector.tensor_add(lo_v, a, bb)
                    nc.vector.tensor_sub(hi_v, a, bb)
                    # chunk k of partition p covers pairs p0 + p*ppp + k*ccpp
                    # + [0, ccpp): DRAM runs of ccpp rows, stride ppp rows.
                    out_eng.dma_start(
                        out=lo[b, p0 : p0 + P * ppp, :]
                        .rearrange("(n r) c -> n (r c)", r=ppp)[
                            :, k * ccpp * C : (k * ccpp + ccpp) * C
                        ],
                        in_=lo_t[:, :],
                    )
                    hi_eng.dma_start(
                        out=hi[b, p0 : p0 + P * ppp, :]
                        .rearrange("(n r) c -> n (r c)", r=ppp)[
                            :, k * ccpp * C : (k * ccpp + ccpp) * C
                        ],
                        in_=hi_t[:, :],
                    )

            for b in range(bl):
                for i in range(ntiles):
                    p0 = i * tile_pairs + 1
                    if i == ntiles - 1:
                        p0 -= 1  # overlap one pair so the tile stays full
                    emit_tile(b, p0, pairs_per_part)

            # pair-0 mini-pass (the causal zero pad): lo[:,0,:] = x[:,0,:],
            # hi[:,0,:] = -x[:,0,:]. Emitted LAST: placed first, its
            # DVE-gated stores stall the SP sequencer before the first bulk
            # load can issue (~2-3us of DMA idle at kernel start).
            t0_in = pool.tile([bl, C], f32, tag="t0in")
            t0_hi = pool.tile([bl, C], f32, tag="t0hi")
            nc.sync.dma_start(out=t0_in[:, :], in_=x[:, 0, :])
            nc.vector.tensor_scalar_mul(t0_hi[:, :], t0_in[:, :], -1.0)
            out_eng.dma_start(out=lo[:, 0, :], in_=t0_in[:, :])
            out_eng.dma_start(out=hi[:, 0, :], in_=t0_hi[:, :])
    return _legalize_waits(nc)


_nc_cache = None


def _get_nc():
    global _nc_cache
    if _nc_cache is None:
        _nc_cache = build()
    return _nc_cache


def run(x, trace=False, trace_cores=None):
    """x: [B, L, C] float32. Returns (lo, hi, BassKernelResults)."""
    from concourse.bass_utils import run_bass_kernel_spmd

    x = np.ascontiguousarray(np.asarray(x), dtype=np.float32)
    assert x.shape == (B, L, C)
    nc = _get_nc()
    in_maps = [{"x": x[i * BL : (i + 1) * BL]} for i in range(NCORES)]
    kwargs = {}
    if trace_cores is not None:
        kwargs["trace_cores"] = trace_cores
    r = run_bass_kernel_spmd(nc, in_maps, list(range(NCORES)), trace=trace, **kwargs)
    lo = np.concatenate([r.results[i]["lo"] for i in range(NCORES)], axis=0)
    hi = np.concatenate([r.results[i]["hi"] for i in range(NCORES)], axis=0)
    return lo, hi, r


def kernel(**inputs):
    lo, hi, _ = run(inputs["x"])
    return lo, hi
